# revision 1
# baseline (speedup 1.0000x reference)
"""MoE (top-2 routed + 2 shared experts, SwiGLU) Trainium2 kernel, 8 NeuronCores.

Sharding:
  - Routed experts: expert-parallel, 2 experts per core (E=16 over 8 cores).
  - Shared experts: H-sharded (each core computes a 256-wide slice of both
    shared experts over ALL tokens); the 0.5 mean factor is folded into w2.
  - Gate: data-parallel over token shards, AllGathered (tiny).
  - Final combine: each core accumulates shared partial + its routed experts'
    scatter-adds into an (N, D) fp32 buffer; ReduceScatter sums across cores;
    host concatenates the 8 reduced shards.

Numerics: FFN matmuls in bf16 with fp32 PSUM accumulation; gate in fp32
(routing decisions are selection-sensitive). Measured ~4e-3 rel error.

Capacity note: reference drops tokens above capacity=ceil(N*K/E*1.25)=2560
per expert. Expected per-expert load is 2048 +/- 44 (binomial), so overflow
is a >11-sigma event; we pad to the same 2560 capacity and never drop.
"""

import numpy as np

B, T, D, H, E, K, S = 4, 4096, 1024, 2048, 16, 2, 2
N = B * T              # 16384 tokens
NCORES = 8
EPC = E // NCORES      # 2 routed experts per core
NSH = N // NCORES      # 2048 tokens per gate shard
CAP = 2560             # per-expert capacity (matches reference)
TBLK = 512             # token block
NB_SH = N // TBLK      # 32 shared blocks
NB_RT = CAP // TBLK    # 5 routed blocks per expert
BIG = 1.0e9            # OOB sentinel for scatter positions

_CACHE = {}


def _build():
    import concourse.bacc as bacc
    import concourse.bass as bass
    import concourse.mybir as mybir
    import concourse.tile as tile
    from concourse.masks import make_upper_triangular

    dt = mybir.dt
    AF = mybir.ActivationFunctionType
    ALU = mybir.AluOpType

    nc = bacc.Bacc("TRN2", target_bir_lowering=False, debug=False,
                   num_devices=NCORES)

    # ---- I/O ----
    xg_d = nc.dram_tensor("xg", [D, NSH], dt.float32, kind="ExternalInput")
    xt_d = nc.dram_tensor("xt", [D, N], dt.bfloat16, kind="ExternalInput")
    xr_d = nc.dram_tensor("xr", [N, D], dt.bfloat16, kind="ExternalInput")
    gw_d = nc.dram_tensor("gw", [D, E], dt.float32, kind="ExternalInput")
    gb_d = nc.dram_tensor("gb", [128, E], dt.float32, kind="ExternalInput")
    es_d = nc.dram_tensor("esel", [EPC, 128, E], dt.float32, kind="ExternalInput")
    s13_d = nc.dram_tensor("sw13", [8, 128, 1024], dt.bfloat16, kind="ExternalInput")
    s2_d = nc.dram_tensor("sw2", [4, 128, 1024], dt.bfloat16, kind="ExternalInput")
    e13_d = nc.dram_tensor("ew13", [EPC, 8, 128, 4096], dt.bfloat16, kind="ExternalInput")
    e2_d = nc.dram_tensor("ew2", [EPC, 16, 128, 1024], dt.bfloat16, kind="ExternalInput")
    out_d = nc.dram_tensor("out", [NSH, D], dt.bfloat16, kind="ExternalOutput")

    RG = [list(range(NCORES))]

    from contextlib import ExitStack
    with tile.TileContext(nc) as tc:
        with ExitStack() as ctx:
            dram = ctx.enter_context(tc.tile_pool(name="dram", bufs=1, space="DRAM"))
            cns = ctx.enter_context(tc.tile_pool(name="const", bufs=1))
            sg = ctx.enter_context(tc.tile_pool(name="gate", bufs=2))
            sxg_g = ctx.enter_context(tc.tile_pool(name="xgt", bufs=2))
            se = ctx.enter_context(tc.tile_pool(name="ext", bufs=2))
            scm = ctx.enter_context(tc.tile_pool(name="cmp", bufs=1))
            sx = ctx.enter_context(tc.tile_pool(name="xts", bufs=2))
            smt = ctx.enter_context(tc.tile_pool(name="mts", bufs=1))
            sy = ctx.enter_context(tc.tile_pool(name="ys", bufs=1))
            syh = ctx.enter_context(tc.tile_pool(name="ysh", bufs=2))
            ssi = ctx.enter_context(tc.tile_pool(name="silu", bufs=2))
            swe = ctx.enter_context(tc.tile_pool(name="wexp", bufs=1))
            sxr = ctx.enter_context(tc.tile_pool(name="gxr", bufs=1))
            psc = ctx.enter_context(tc.tile_pool(name="psc", bufs=2, space="PSUM"))
            psh = ctx.enter_context(tc.tile_pool(name="psh", bufs=4, space="PSUM"))
            psy = ctx.enter_context(tc.tile_pool(name="psy", bufs=2, space="PSUM"))
            # ---------- DRAM temporaries ----------
            ag_in = dram.tile([NSH, 2 * E], dt.float32)
            ag_out = dram.tile([N, 2 * E], dt.float32, addr_space="Shared")
            pairs = [dram.tile([CAP, 2], dt.float32, name=f"pairs{i}")
                     for i in range(EPC)]
            rbuf = dram.tile([N, D], dt.bfloat16)
            rs_out = dram.tile([NSH, D], dt.bfloat16)

            # ---------- constants ----------
            gw_sb = cns.tile([128, 8, E], dt.float32)
            nc.sync.dma_start(gw_sb[:], gw_d.rearrange("(c p) e -> p c e", p=128))
            gb_sb = cns.tile([128, E], dt.float32)
            nc.sync.dma_start(gb_sb[:], gb_d[:])
            es_sb = cns.tile([128, EPC, E], dt.float32)
            nc.sync.dma_start(es_sb[:], es_d.rearrange("l p e -> p l e"))
            su = cns.tile([128, 128], dt.float32)
            make_upper_triangular(nc, su[:], val=1.0, diag=False)  # 1 iff row < col
            ones_col = cns.tile([128, 1], dt.float32)
            nc.vector.memset(ones_col[:], 1.0)
            tok_i = cns.tile([128, 128], dt.int32)
            nc.gpsimd.iota(tok_i[:], pattern=[[128, 128]], base=0,
                           channel_multiplier=1)
            tok_f = cns.tile([128, 128], dt.float32)
            nc.vector.tensor_copy(tok_f[:], tok_i[:])
            s13_sb = cns.tile([128, 8, 1024], dt.bfloat16)
            nc.sync.dma_start(s13_sb[:], s13_d.rearrange("c p h -> p c h"))
            s2_sb = cns.tile([128, 4, 1024], dt.bfloat16)
            nc.sync.dma_start(s2_sb[:], s2_d.rearrange("c p h -> p c h"))
            wslab = cns.tile([128, EPC, 128], dt.float32)
            mslab = cns.tile([128, EPC, 128], dt.float32)
            idx16 = cns.tile([128, EPC, CAP // 16], dt.int16)
            wsc = cns.tile([128, EPC, CAP // 128], dt.float32)

            def shared_block(blk):
                    xtb = sx.tile([128, 8, TBLK], dt.bfloat16)
                    nc.sync.dma_start(
                        xtb[:],
                        xt_d.rearrange("(c p) n -> p c n", p=128)[
                            :, :, blk * TBLK:(blk + 1) * TBLK])
                    mts = smt.tile([128, 4, TBLK], dt.bfloat16, tag="mt", padded_shape=[128, 16, TBLK])
                    for hp in range(4):
                        ph1 = psh.tile([128, TBLK], dt.float32, tag="ph")
                        ph3 = psh.tile([128, TBLK], dt.float32, tag="ph")
                        for dc in range(8):
                            nc.tensor.matmul(
                                ph1[:], lhsT=s13_sb[:, dc, hp * 128:(hp + 1) * 128],
                                rhs=xtb[:, dc, :], start=(dc == 0), stop=(dc == 7))
                        for dc in range(8):
                            nc.tensor.matmul(
                                ph3[:], lhsT=s13_sb[:, dc, 512 + hp * 128:512 + (hp + 1) * 128],
                                rhs=xtb[:, dc, :], start=(dc == 0), stop=(dc == 7))
                        sil = ssi.tile([128, TBLK], dt.float32)
                        nc.scalar.activation(sil[:], ph1[:], AF.Silu)
                        nc.vector.tensor_mul(mts[:, hp, :], sil[:], ph3[:])
                    for t4 in range(4):
                        row0 = blk * TBLK + t4 * 128
                        ysh = syh.tile([128, D], dt.bfloat16, tag="ysh")
                        for dh in range(2):
                            py = psy.tile([128, 512], dt.float32)
                            for hp in range(4):
                                nc.tensor.matmul(
                                    py[:], lhsT=mts[:, hp, t4 * 128:(t4 + 1) * 128],
                                    rhs=s2_sb[:, hp, dh * 512:(dh + 1) * 512],
                                    start=(hp == 0), stop=(hp == 3))
                            nc.vector.tensor_copy(ysh[:, dh * 512:(dh + 1) * 512], py[:])
                        nc.sync.dma_start(rbuf[row0:row0 + 128, :], ysh[:])


            # ---------- P1: gate on local token shard ----------
            for tb in range(NSH // 128):
                xgt = sxg_g.tile([128, 8, 128], dt.float32)
                nc.sync.dma_start(
                    xgt[:],
                    xg_d.rearrange("(c p) n -> p c n", p=128)[
                        :, :, tb * 128:(tb + 1) * 128],
                )
                pg = psc.tile([128, E], dt.float32, tag="pc")
                for dc in range(8):
                    nc.tensor.matmul(pg[:], lhsT=xgt[:, dc, :], rhs=gw_sb[:, dc, :],
                                     start=(dc == 0), stop=(dc == 7))
                logits = sg.tile([128, E], dt.float32)
                nc.vector.tensor_copy(logits[:], pg[:])
                mx8 = sg.tile([128, 8], dt.float32)
                nc.vector.max(mx8[:], logits[:])
                negmx = sg.tile([128, 1], dt.float32)
                nc.vector.tensor_scalar(negmx[:], mx8[:, 0:1], -1.0, None,
                                        op0=ALU.mult)
                exps = sg.tile([128, E], dt.float32)
                nc.scalar.activation(exps[:], logits[:], AF.Exp,
                                     bias=negmx[:, 0:1], scale=1.0)
                ssum = sg.tile([128, 1], dt.float32)
                nc.vector.tensor_reduce(ssum[:], exps[:], axis=mybir.AxisListType.X,
                                        op=ALU.add)
                rcp = sg.tile([128, 1], dt.float32)
                nc.vector.reciprocal(rcp[:], ssum[:])
                scores = sg.tile([128, E], dt.float32)
                nc.vector.tensor_scalar(scores[:], exps[:], rcp[:, 0:1], None,
                                        op0=ALU.mult)
                nc.vector.tensor_add(scores[:], scores[:], gb_sb[:])
                smax = sg.tile([128, 8], dt.float32)
                nc.vector.max(smax[:], scores[:])
                mask = sg.tile([128, E], dt.float32)
                nc.vector.tensor_tensor(
                    out=mask[:], in0=scores[:],
                    in1=smax[:, 1:2].to_broadcast([128, E]), op=ALU.is_ge)
                wmat = sg.tile([128, E], dt.float32)
                nc.vector.tensor_mul(wmat[:], logits[:], mask[:])
                nc.sync.dma_start(ag_in[tb * 128:(tb + 1) * 128, 0:E], wmat[:])
                nc.sync.dma_start(ag_in[tb * 128:(tb + 1) * 128, E:2 * E], mask[:])

            # ---------- P2: AllGather routing info ----------
            nc.gpsimd.collective_compute(
                "AllGather", ALU.bypass, replica_groups=RG,
                ins=[ag_in[:]], outs=[ag_out[:]])

            # ---------- P5a: first shared blocks (keep PE busy during AG) ----
            HOIST = 2
            for blk in range(HOIST):
                shared_block(blk)

            # ---------- P3: extract local-expert weight/mask slabs ----------
            for t in range(N // 128):
                wm = se.tile([128, 2 * E], dt.float32)
                nc.sync.dma_start(wm[:], ag_out[t * 128:(t + 1) * 128, :])
                for le in range(EPC):
                    tmpw = se.tile([128, E], dt.float32)
                    nc.vector.tensor_mul(tmpw[:], wm[:, 0:E], es_sb[:, le, :])
                    nc.vector.tensor_reduce(wslab[:, le, t:t + 1], tmpw[:],
                                            axis=mybir.AxisListType.X, op=ALU.add)
                    tmpm = se.tile([128, E], dt.float32)
                    nc.vector.tensor_mul(tmpm[:], wm[:, E:2 * E], es_sb[:, le, :])
                    nc.vector.tensor_reduce(mslab[:, le, t:t + 1], tmpm[:],
                                            axis=mybir.AxisListType.X, op=ALU.add)

            # ---------- P4: compaction (positions + scatter of (tok, w)) ----------
            for le in range(EPC):
                pcs = psc.tile([128, 1], dt.float32, tag="pc")
                nc.tensor.matmul(pcs[:], lhsT=mslab[:, le, :], rhs=ones_col[:],
                                 start=True, stop=True)
                csum = scm.tile([128, 1], dt.float32)
                nc.vector.tensor_copy(csum[:], pcs[:])
                pos = psc.tile([128, 128], dt.float32, tag="pc")
                # pos[p,t] = sum_{c<t} csum[c] + sum_{p'<p} mask[p',t]
                nc.tensor.matmul(pos[:], lhsT=csum[:, 0:1].to_broadcast([128, 128]),
                                 rhs=su[:], start=True, stop=False)
                nc.tensor.matmul(pos[:], lhsT=su[:], rhs=mslab[:, le, :],
                                 start=False, stop=True)
                bigm = scm.tile([128, 128], dt.float32)
                nc.vector.tensor_scalar(bigm[:], mslab[:, le, :], -BIG, BIG,
                                        op0=ALU.mult, op1=ALU.add)
                posv = scm.tile([128, 128], dt.float32)
                nc.vector.tensor_mul(posv[:], pos[:], mslab[:, le, :])
                posf = scm.tile([128, 128], dt.float32)
                nc.vector.tensor_add(posf[:], posv[:], bigm[:])
                offs = scm.tile([128, 128], dt.int32)
                nc.vector.tensor_copy(offs[:], posf[:])
                wtok = scm.tile([128, 128, 2], dt.float32)
                nc.vector.tensor_copy(wtok[:, :, 0], tok_f[:])
                nc.vector.tensor_copy(wtok[:, :, 1], wslab[:, le, :])
                zb = scm.tile([128, CAP // 128, 2], dt.float32)
                nc.vector.memset(zb[:], 0.0)
                nc.sync.dma_start(
                    pairs[le].rearrange("(c p) e -> p c e", p=128), zb[:])
                for t in range(128):
                    nc.gpsimd.indirect_dma_start(
                        out=pairs[le][:],
                        out_offset=bass.IndirectOffsetOnAxis(
                            ap=offs[:, t:t + 1], axis=0),
                        in_=wtok[:, t, :], in_offset=None,
                        bounds_check=CAP - 1, oob_is_err=False)

                # wrapped int16 index table (16-wrap, replicated to 8 stripes)
                idxf = scm.tile([128, CAP // 16], dt.float32)
                for k in range(8):
                    nc.sync.dma_start(
                        idxf[16 * k:16 * (k + 1), :],
                        pairs[le].rearrange("(c s) e -> s c e", s=16)[:, :, 0])
                nc.vector.tensor_copy(idx16[:, le, :], idxf[:])
                nc.sync.dma_start(
                    wsc[:, le, :],
                    pairs[le].rearrange("(c p) e -> p c e", p=128)[:, :, 1])

            # Preload expert-0 weights early so the DMA overlaps the shared
            # phase; per-chunk tiles so expert-1's loads start as soon as
            # expert 0 finishes reading each chunk (per-chunk WAR).
            def load_expert_w(le):
                e13c = []
                for dc in range(8):
                    t13 = swe.tile([128, 4096], dt.bfloat16, tag=f"e13_{dc}",
                                   name=f"e13c{le}_{dc}")
                    nc.sync.dma_start(t13[:], e13_d[le, dc])
                    e13c.append(t13)
                e2c = []
                for hb in range(16):
                    t2 = swe.tile([128, 1024], dt.bfloat16, tag=f"e2_{hb}",
                                  name=f"e2c{le}_{hb}")
                    nc.sync.dma_start(t2[:], e2_d[le, hb])
                    e2c.append(t2)
                return e13c, e2c

            ew_p = load_expert_w(0)

            # ---------- P5: shared experts (H-sharded, all tokens) ----------
            for blk in range(HOIST, NB_SH):
                shared_block(blk)

            # ---------- P6: routed experts ----------
            for le in range(EPC):
                e13c, e2c = ew_p if le == 0 else load_expert_w(le)
                for blk in range(NB_RT):
                    xgT = sxr.tile([128, 8, TBLK], dt.bfloat16)
                    nc.gpsimd.dma_gather(
                        out_ap=xgT[:], in_ap=xr_d[:],
                        idxs_ap=idx16[:, le, blk * 32:(blk + 1) * 32],
                        num_idxs=TBLK, num_idxs_reg=TBLK,
                        elem_size=D, transpose=True)
                    mtr = smt.tile([128, 16, TBLK], dt.bfloat16, tag="mt")
                    for hb in range(16):
                        ph1 = psh.tile([128, TBLK], dt.float32, tag="ph")
                        ph3 = psh.tile([128, TBLK], dt.float32, tag="ph")
                        for dc in range(8):
                            nc.tensor.matmul(
                                ph1[:], lhsT=e13c[dc][:, hb * 128:(hb + 1) * 128],
                                rhs=xgT[:, dc, :], start=(dc == 0), stop=(dc == 7))
                        for dc in range(8):
                            nc.tensor.matmul(
                                ph3[:], lhsT=e13c[dc][:, 2048 + hb * 128:2048 + (hb + 1) * 128],
                                rhs=xgT[:, dc, :], start=(dc == 0), stop=(dc == 7))
                        sil = ssi.tile([128, TBLK], dt.float32)
                        nc.scalar.activation(sil[:], ph1[:], AF.Silu)
                        nc.vector.tensor_mul(mtr[:, hb, :], sil[:], ph3[:])
                    ysb = sy.tile([128, 4, D], dt.bfloat16)
                    for t4 in range(4):
                        wcol = wsc[:, le, blk * 4 + t4:blk * 4 + t4 + 1]
                        for dh in range(2):
                            py = psy.tile([128, 512], dt.float32)
                            for hb in range(16):
                                nc.tensor.matmul(
                                    py[:], lhsT=mtr[:, hb, t4 * 128:(t4 + 1) * 128],
                                    rhs=e2c[hb][:, dh * 512:(dh + 1) * 512],
                                    start=(hb == 0), stop=(hb == 15))
                            nc.vector.tensor_scalar(
                                ysb[:, t4, dh * 512:(dh + 1) * 512], py[:],
                                wcol, None, op0=ALU.mult)
                    nc.gpsimd.dma_scatter_add(
                        out_ap=rbuf[:], in_ap=ysb[:],
                        idxs_ap=idx16[:, le, blk * 32:(blk + 1) * 32],
                        num_idxs=TBLK, num_idxs_reg=TBLK, elem_size=D)

            # ---------- P7: ReduceScatter + output ----------
            nc.gpsimd.collective_compute(
                "ReduceScatter", ALU.add, replica_groups=RG,
                ins=[rbuf[:]], outs=[rs_out[:]])
            nc.sync.dma_start(out_d[:], rs_out[:])

    nc.compile()
    return nc


def _prep_inputs(inputs):
    import ml_dtypes
    bf16 = ml_dtypes.bfloat16

    x = np.ascontiguousarray(np.asarray(inputs["x"], np.float32).reshape(N, D))
    gw = np.asarray(inputs["gate_w"], np.float32)
    gb = np.asarray(inputs["gate_b"], np.float32)
    ew1 = np.asarray(inputs["ew1"], np.float32)
    ew3 = np.asarray(inputs["ew3"], np.float32)
    ew2 = np.asarray(inputs["ew2"], np.float32)
    sw1 = np.asarray(inputs["sw1"], np.float32)
    sw3 = np.asarray(inputs["sw3"], np.float32)
    sw2 = np.asarray(inputs["sw2"], np.float32)

    xt = np.ascontiguousarray(x.T).astype(bf16)               # (D, N)
    xr = x.astype(bf16)                                       # (N, D)
    gb_b = np.broadcast_to(gb, (128, E)).copy()

    in_maps = []
    for c in range(NCORES):
        hs = slice(c * (H // NCORES), (c + 1) * (H // NCORES))
        # shared cat: [w1_s0 | w1_s1 | w3_s0 | w3_s1] (1024, 1024)
        s13 = np.concatenate(
            [sw1[0][:, hs], sw1[1][:, hs], sw3[0][:, hs], sw3[1][:, hs]], axis=1)
        s13 = s13.reshape(8, 128, 1024).astype(bf16)
        # shared w2 cat rows [w2_s0 ; w2_s1] * 0.5  -> (512, 1024) -> (4,128,1024)
        s2 = (np.concatenate([sw2[0][hs, :], sw2[1][hs, :]], axis=0) * 0.5)
        s2 = s2.reshape(4, 128, 1024).astype(bf16)
        e13 = np.empty((EPC, 8, 128, 4096), np.float32)
        e2c = np.empty((EPC, 16, 128, 1024), np.float32)
        esel = np.zeros((EPC, 128, E), np.float32)
        for le in range(EPC):
            ei = c * EPC + le
            cat = np.concatenate([ew1[ei], ew3[ei]], axis=1)  # (1024, 4096)
            e13[le] = cat.reshape(8, 128, 4096)
            e2c[le] = ew2[ei].reshape(16, 128, 1024)
            esel[le, :, ei] = 1.0
        xg = np.ascontiguousarray(x[c * NSH:(c + 1) * NSH].T)  # (D, NSH) fp32
        in_maps.append({
            "xg": xg, "xt": xt, "xr": xr, "gw": gw, "gb": gb_b,
            "esel": esel, "sw13": s13, "sw2": s2,
            "ew13": e13.astype(bf16), "ew2": e2c.astype(bf16),
        })
    return in_maps


def kernel(**inputs):
    from concourse.bass_utils import run_bass_kernel_spmd

    if "nc" not in _CACHE:
        _CACHE["nc"] = _build()
    nc = _CACHE["nc"]
    in_maps = _prep_inputs(inputs)
    res = run_bass_kernel_spmd(nc, in_maps, core_ids=list(range(NCORES)))
    _CACHE["last_result"] = res
    out = np.concatenate([res.results[c]["out"] for c in range(NCORES)], axis=0)
    return out.astype(np.float32).reshape(B, T, D)

